# revision 21
# baseline (speedup 1.0000x reference)
"""Trainium2 kernel for nn_BranchModel_9680856285960 (moe_routing).

Math: the reference scatters per-branch sparse weights into dense
(n_br, n_out, n_in) tensors, einsums against x, then takes a context-
gated masked sum over branches followed by relu.  Because the mask-
weighted branch sum commutes with the contraction over input features,
the whole model collapses to a 3-layer dense MLP

    out = relu(relu(x @ Weff1.T) @ Weff2.T) @ W3 + b3

where  Weff_l[o, i] = sum_{r,k} masks_l[ctx, r, o] * w_l[r, o, k]
                                * [idx_l[r, o, k] == i].

The effective-weight fold (a scatter-add over 5.6M index/value pairs) is
data-dependent element-granular addressing, which Trainium2 has no fast
engine for; it is done once on the host here, and the device then runs
the dense pipeline.  Weights/activations stream as fp16 (the kernel is
HBM-bound on the weight stream; fp16 halves it and runs the PE at full
rate with fp32 PSUM accumulation).

Sharding: data-parallel over batch (8 cores x 128 rows), effective
weights replicated per core, activations kept feature-major on chip.
No collectives.
"""

import os
import sys
import numpy as np

for _p in ("/opt/trn_rl_repo",):
    if os.path.isdir(_p) and _p not in sys.path:
        sys.path.append(_p)

from contextlib import ExitStack

from concourse import bass, mybir
import concourse.bacc as bacc
import concourse.tile as tile
from concourse.bass_utils import run_bass_kernel_spmd
from concourse.masks import make_identity

F32 = mybir.dt.float32
F16 = mybir.dt.float16

BATCH, NIN, NH, NOUT = 1024, 784, 2000, 10
NCORES = 8
BS = BATCH // NCORES            # 128 batch rows per core
P = 128


def _tiles(total, step):
    out, o = [], 0
    while o < total:
        out.append((o, min(step, total - o)))
        o += step
    return out


MT1 = _tiles(NIN, P)            # layer-1 contraction tiles: 6x128 + 16
MT2 = _tiles(NH, P)             # layer-2/3 contraction tiles: 15x128 + 80
NCHK = _tiles(NH, 512)          # psum output chunks: 3x512 + 464

# Exposed for the test harness: the BassKernelResults of the last run.
LAST_RESULT = None
_CACHE = {}


def _build_weff(w, idx, mask_row, n_in):
    """Fold masks + branch sum into a dense effective weight matrix.

    Weff[o, i] = sum_{r,k} mask_row[r, o] * w[r, o, k] * [idx[r, o, k] == i]
    """
    n_br, n_out, npb = w.shape
    acc = np.zeros(n_out * n_in, np.float64)
    base = (np.arange(n_out, dtype=np.int64) * n_in)[:, None]
    for r in range(n_br):
        flat = (base + idx[r].astype(np.int64)).ravel()
        vals = (w[r].astype(np.float64) * mask_row[r].astype(np.float64)[:, None]).ravel()
        acc += np.bincount(flat, weights=vals, minlength=n_out * n_in)
    return acc.reshape(n_out, n_in).astype(np.float32)


def _mlp_body(tc, xT, w1t, w2t, w3p, b3r, out):
    nc = tc.nc
    rings = [nc.sync, nc.scalar]          # the two HWDGE rings

    with ExitStack() as ctx:
        const = ctx.enter_context(tc.tile_pool(name="const", bufs=1))
        wp = ctx.enter_context(tc.tile_pool(name="wslab", bufs=1))
        act = ctx.enter_context(tc.tile_pool(name="act", bufs=1))
        pacc = ctx.enter_context(tc.tile_pool(name="pacc", bufs=1, space="PSUM"))
        ptr = ctx.enter_context(tc.tile_pool(name="ptr", bufs=1, space="PSUM"))

        ident = const.tile([P, P], F16, tag="ident")
        make_identity(nc, ident[:])

        # All 7 layer-1 x tiles in one packed DMA: xbig[p, t, b] = xT[t*128+p, b]
        xbig = const.tile([P, len(MT1), P], F16, tag="xbig")
        nc.gpsimd.dma_start(
            out=xbig[:], in_=xT.rearrange("(t p) b -> p t b", p=P))
        xts = [xbig[:sz, t, :] for t, (off, sz) in enumerate(MT1)]

        b3t = const.tile([P, NOUT], F32, tag="b3")
        nc.gpsimd.dma_start(out=b3t[:], in_=b3r)

        # W3 host-packed as [128 partitions, 16 contraction tiles, 10]
        w3t = const.tile([P, len(MT2), NOUT], F16, tag="w3")
        nc.gpsimd.dma_start(out=w3t[:], in_=w3p)

        # Stream all weight slabs up front (they all fit in SBUF); the two
        # HWDGE rings run in parallel and the PE consumes slabs as they land.
        w1s, w2s = [], []
        for t, (off, sz) in enumerate(MT1):
            slab = wp.tile([sz, NH], F16, name=f"w1s{t}", tag=f"w1s{t}")
            rings[t % 2].dma_start(out=slab[:], in_=w1t[off:off + sz, :])
            w1s.append(slab[:])
        for t, (off, sz) in enumerate(MT2):
            slab = wp.tile([sz, NH], F16, name=f"w2s{t}", tag=f"w2s{t}")
            rings[t % 2].dma_start(out=slab[:], in_=w2t[off:off + sz, :])
            w2s.append(slab[:])

        # ---- Layer 1: H1 = relu(x @ Weff1.T), batch on partitions
        h1 = act.tile([P, NH], F16, tag="h1")
        ps1 = [pacc.tile([P, sz], F32, name=f"ps1_{n}", tag=f"ps{n}")
               for n, (_, sz) in enumerate(NCHK)]
        for t in range(len(MT1)):
            for n, (noff, nsz) in enumerate(NCHK):
                nc.tensor.matmul(
                    ps1[n][:],
                    lhsT=xts[t],
                    rhs=w1s[t][:, noff:noff + nsz],
                    start=(t == 0),
                    stop=(t == len(MT1) - 1),
                )
        for n, (noff, nsz) in enumerate(NCHK):
            nc.vector.tensor_scalar_max(h1[:, noff:noff + nsz], ps1[n][:], 0.0)

        # ---- Layer 2: H2 = relu(H1 @ Weff2.T).  Each contraction tile t is
        # transposed out of H1 and consumed immediately, so PE program order
        # doesn't stall layer-2 matmuls behind all 16 transposes.
        pts = [ptr.tile([P, P], F16, name=f"pt{i}", tag=f"pt{i}")
               for i in range(3)]
        h1Tb = act.tile([P, len(MT2), P], F16, tag="h1Tb")
        h2 = act.tile([P, NH], F16, tag="h2")
        ps2 = [pacc.tile([P, sz], F32, name=f"ps2_{n}", tag=f"ps{n}")
               for n, (_, sz) in enumerate(NCHK)]
        for t, (off, sz) in enumerate(MT2):
            pt = pts[t % 3]
            nc.tensor.transpose(pt[:sz, :], h1[:, off:off + sz], ident[:])
            nc.vector.tensor_copy(h1Tb[:sz, t, :], pt[:sz, :])
            for n, (noff, nsz) in enumerate(NCHK):
                nc.tensor.matmul(
                    ps2[n][:],
                    lhsT=h1Tb[:sz, t, :],
                    rhs=w2s[t][:, noff:noff + nsz],
                    start=(t == 0),
                    stop=(t == len(MT2) - 1),
                )
        # Per-j-tile relu (alternating DVE/ACT) so each transpose can start
        # as soon as its 128 columns are ready — this chain is the kernel tail.
        for j, (off, sz) in enumerate(MT2):
            n = j // 4
            csl = slice(off - NCHK[n][0], off - NCHK[n][0] + sz)
            if j % 2 == 0:
                nc.vector.tensor_scalar_max(h2[:, off:off + sz],
                                            ps2[n][:, csl], 0.0)
            else:
                nc.scalar.activation(h2[:, off:off + sz], ps2[n][:, csl],
                                     mybir.ActivationFunctionType.Relu)

        # Transpose H2 for the layer-3 contraction (copies split DVE/ACT to
        # shorten the end-of-kernel critical path)
        h2Tb = act.tile([P, len(MT2), P], F16, tag="h2Tb")
        h2T = []
        for j, (off, sz) in enumerate(MT2):
            pt = pts[j % 3]
            nc.tensor.transpose(pt[:sz, :], h2[:, off:off + sz], ident[:])
            if j % 4 == 3:
                nc.scalar.copy(h2Tb[:sz, j, :], pt[:sz, :])
            else:
                nc.vector.tensor_copy(h2Tb[:sz, j, :], pt[:sz, :])
            h2T.append(h2Tb[:sz, j, :])

        # ---- Layer 3: out = H2 @ W3 + b3
        ps3 = pacc.tile([P, NOUT], F32, tag="ps3")
        for t, (off, sz) in enumerate(MT2):
            nc.tensor.matmul(
                ps3[:],
                lhsT=h2T[t],
                rhs=w3t[:sz, t, :],
                start=(t == 0),
                stop=(t == len(MT2) - 1),
            )
        o = act.tile([P, NOUT], F32, tag="o")
        nc.vector.tensor_add(o[:], ps3[:], b3t[:])
        nc.sync.dma_start(out=out, in_=o[:])


def _get_program():
    if "nc" in _CACHE:
        return _CACHE["nc"]
    nc = bacc.Bacc("TRN2", target_bir_lowering=False, debug=False,
                   enable_asserts=False, num_devices=NCORES)
    xT = nc.dram_tensor("xT", [len(MT1) * P, BS], F16,
                        kind="ExternalInput").ap()
    w1t = nc.dram_tensor("w1t", [NIN, NH], F16, kind="ExternalInput").ap()
    w2t = nc.dram_tensor("w2t", [NH, NH], F16, kind="ExternalInput").ap()
    w3p = nc.dram_tensor("w3p", [P, len(MT2), NOUT], F16,
                         kind="ExternalInput").ap()
    b3r = nc.dram_tensor("b3r", [P, NOUT], F32, kind="ExternalInput").ap()
    out = nc.dram_tensor("out", [BS, NOUT], F32, kind="ExternalOutput").ap()
    with tile.TileContext(nc) as tc:
        _mlp_body(tc, xT, w1t, w2t, w3p, b3r, out)
    nc.compile()
    _CACHE["nc"] = nc
    return nc


def kernel(x, w1, idx1, w2, idx2, masks1, masks2, W3, b3, context):
    global LAST_RESULT
    x = np.ascontiguousarray(np.asarray(x, dtype=np.float32))
    ctxi = int(np.asarray(context))

    weff1 = _build_weff(np.asarray(w1), np.asarray(idx1),
                        np.asarray(masks1)[ctxi], NIN)
    weff2 = _build_weff(np.asarray(w2), np.asarray(idx2),
                        np.asarray(masks2)[ctxi], NH)
    w1t = np.ascontiguousarray(weff1.T.astype(np.float16))    # (784, 2000)
    w2t = np.ascontiguousarray(weff2.T.astype(np.float16))    # (2000, 2000)

    # W3 packed to [128, n_tiles, 10]: w3p[m, t, :] = W3[t*128 + m, :]
    w3f = np.asarray(W3).astype(np.float16)
    w3p = np.zeros((P, len(MT2), NOUT), np.float16)
    for t, (off, sz) in enumerate(MT2):
        w3p[:sz, t, :] = w3f[off:off + sz, :]
    b3r = np.ascontiguousarray(
        np.broadcast_to(np.asarray(b3, dtype=np.float32), (P, NOUT)).copy())

    try:
        import antenv.axon_hooks  # noqa: F401
    except Exception:
        os.environ.setdefault("BASS_NEVER_TRACE", "1")

    nc = _get_program()
    in_maps = []
    for c in range(NCORES):
        xTc = np.zeros((len(MT1) * P, BS), np.float16)
        xTc[:NIN] = x[c * BS:(c + 1) * BS].T.astype(np.float16)
        xT = xTc
        in_maps.append({"xT": xT, "w1t": w1t, "w2t": w2t, "w3p": w3p,
                        "b3r": b3r})

    LAST_RESULT = run_bass_kernel_spmd(nc, in_maps, list(range(NCORES)))
    return np.concatenate(
        [LAST_RESULT.results[c]["out"] for c in range(NCORES)], axis=0)


# revision 22
# speedup vs baseline: 1.0192x; 1.0192x over previous
"""Trainium2 kernel for nn_BranchModel_9680856285960 (moe_routing).

Math: the reference scatters per-branch sparse weights into dense
(n_br, n_out, n_in) tensors, einsums against x, then takes a context-
gated masked sum over branches followed by relu.  Because the mask-
weighted branch sum commutes with the contraction over input features,
the whole model collapses to a 3-layer dense MLP

    out = relu(relu(x @ Weff1.T) @ Weff2.T) @ W3 + b3

where  Weff_l[o, i] = sum_{r,k} masks_l[ctx, r, o] * w_l[r, o, k]
                                * [idx_l[r, o, k] == i].

The effective-weight fold (a scatter-add over 5.6M index/value pairs) is
data-dependent element-granular addressing, which Trainium2 has no fast
engine for; it is done once on the host here, and the device then runs
the dense pipeline.  Weights/activations stream as fp16 (the kernel is
HBM-bound on the weight stream; fp16 halves it and runs the PE at full
rate with fp32 PSUM accumulation).

Sharding: data-parallel over batch (8 cores x 128 rows), effective
weights replicated per core, activations kept feature-major on chip.
No collectives.
"""

import os
import sys
import numpy as np

for _p in ("/opt/trn_rl_repo",):
    if os.path.isdir(_p) and _p not in sys.path:
        sys.path.append(_p)

from contextlib import ExitStack

from concourse import bass, mybir
import concourse.bacc as bacc
import concourse.tile as tile
from concourse.bass_utils import run_bass_kernel_spmd
from concourse.masks import make_identity

F32 = mybir.dt.float32
F16 = mybir.dt.float16

BATCH, NIN, NH, NOUT = 1024, 784, 2000, 10
NCORES = 8
BS = BATCH // NCORES            # 128 batch rows per core
P = 128


def _tiles(total, step):
    out, o = [], 0
    while o < total:
        out.append((o, min(step, total - o)))
        o += step
    return out


MT1 = _tiles(NIN, P)            # layer-1 contraction tiles: 6x128 + 16
MT2 = _tiles(NH, P)             # layer-2/3 contraction tiles: 15x128 + 80
NCHK = _tiles(NH, 512)          # psum output chunks: 3x512 + 464

# Exposed for the test harness: the BassKernelResults of the last run.
LAST_RESULT = None
_CACHE = {}


def _build_weff(w, idx, mask_row, n_in):
    """Fold masks + branch sum into a dense effective weight matrix.

    Weff[o, i] = sum_{r,k} mask_row[r, o] * w[r, o, k] * [idx[r, o, k] == i]
    """
    n_br, n_out, npb = w.shape
    acc = np.zeros(n_out * n_in, np.float64)
    base = (np.arange(n_out, dtype=np.int64) * n_in)[:, None]
    for r in range(n_br):
        flat = (base + idx[r].astype(np.int64)).ravel()
        vals = (w[r].astype(np.float64) * mask_row[r].astype(np.float64)[:, None]).ravel()
        acc += np.bincount(flat, weights=vals, minlength=n_out * n_in)
    return acc.reshape(n_out, n_in).astype(np.float32)


def _mlp_body(tc, xT, w1t, w2t, w3p, b3r, out):
    nc = tc.nc
    rings = [nc.sync, nc.scalar]          # the two HWDGE rings

    with ExitStack() as ctx:
        const = ctx.enter_context(tc.tile_pool(name="const", bufs=1))
        wp = ctx.enter_context(tc.tile_pool(name="wslab", bufs=1))
        act = ctx.enter_context(tc.tile_pool(name="act", bufs=1))
        pacc = ctx.enter_context(tc.tile_pool(name="pacc", bufs=1, space="PSUM"))
        ptr = ctx.enter_context(tc.tile_pool(name="ptr", bufs=1, space="PSUM"))

        ident = const.tile([P, P], F16, tag="ident")
        make_identity(nc, ident[:])

        # All 7 layer-1 x tiles in one packed DMA: xbig[p, t, b] = xT[t*128+p, b]
        xbig = const.tile([P, len(MT1), P], F16, tag="xbig")
        nc.gpsimd.dma_start(
            out=xbig[:], in_=xT.rearrange("(t p) b -> p t b", p=P))
        xts = [xbig[:sz, t, :] for t, (off, sz) in enumerate(MT1)]

        b3t = const.tile([P, NOUT], F32, tag="b3")
        nc.gpsimd.dma_start(out=b3t[:], in_=b3r)

        # W3 host-packed as [128 partitions, 16 contraction tiles, 10]
        w3t = const.tile([P, len(MT2), NOUT], F16, tag="w3")
        nc.gpsimd.dma_start(out=w3t[:], in_=w3p)

        # Stream all weight slabs up front (they all fit in SBUF); the two
        # HWDGE rings run in parallel and the PE consumes slabs as they land.
        w1s, w2s = [], []
        for t, (off, sz) in enumerate(MT1):
            slab = wp.tile([sz, NH], F16, name=f"w1s{t}", tag=f"w1s{t}")
            rings[t % 2].dma_start(out=slab[:], in_=w1t[off:off + sz, :])
            w1s.append(slab[:])
        for t, (off, sz) in enumerate(MT2):
            slab = wp.tile([sz, NH], F16, name=f"w2s{t}", tag=f"w2s{t}")
            rings[t % 2].dma_start(out=slab[:], in_=w2t[off:off + sz, :])
            w2s.append(slab[:])

        # ---- Layer 1: H1 = relu(x @ Weff1.T), batch on partitions
        h1 = act.tile([P, NH], F16, tag="h1")
        ps1 = [pacc.tile([P, sz], F32, name=f"ps1_{n}", tag=f"ps{n}")
               for n, (_, sz) in enumerate(NCHK)]
        for t in range(len(MT1)):
            for n, (noff, nsz) in enumerate(NCHK):
                nc.tensor.matmul(
                    ps1[n][:],
                    lhsT=xts[t],
                    rhs=w1s[t][:, noff:noff + nsz],
                    start=(t == 0),
                    stop=(t == len(MT1) - 1),
                )
        for n, (noff, nsz) in enumerate(NCHK):
            nc.vector.tensor_scalar_max(h1[:, noff:noff + nsz], ps1[n][:], 0.0)

        # Transpose H1 to feature-major tiles for the layer-2 contraction
        pts = [ptr.tile([P, P], F16, name=f"pt{i}", tag=f"pt{i}")
               for i in range(3)]
        h1Tb = act.tile([P, len(MT2), P], F16, tag="h1Tb")
        h1T = []
        for j, (off, sz) in enumerate(MT2):
            pt = pts[j % 3]
            nc.tensor.transpose(pt[:sz, :], h1[:, off:off + sz], ident[:])
            nc.vector.tensor_copy(h1Tb[:sz, j, :], pt[:sz, :])
            h1T.append(h1Tb[:sz, j, :])

        # ---- Layer 2: H2 = relu(H1 @ Weff2.T)
        h2 = act.tile([P, NH], F16, tag="h2")
        ps2 = [pacc.tile([P, sz], F32, name=f"ps2_{n}", tag=f"ps{n}")
               for n, (_, sz) in enumerate(NCHK)]
        for t in range(len(MT2)):
            for n, (noff, nsz) in enumerate(NCHK):
                nc.tensor.matmul(
                    ps2[n][:],
                    lhsT=h1T[t],
                    rhs=w2s[t][:, noff:noff + nsz],
                    start=(t == 0),
                    stop=(t == len(MT2) - 1),
                )
        # Per-j-tile relu (alternating DVE/ACT) so each transpose can start
        # as soon as its 128 columns are ready — this chain is the kernel tail.
        for j, (off, sz) in enumerate(MT2):
            n = j // 4
            csl = slice(off - NCHK[n][0], off - NCHK[n][0] + sz)
            if j % 2 == 0:
                nc.vector.tensor_scalar_max(h2[:, off:off + sz],
                                            ps2[n][:, csl], 0.0)
            else:
                nc.scalar.activation(h2[:, off:off + sz], ps2[n][:, csl],
                                     mybir.ActivationFunctionType.Relu)

        # Transpose H2 for the layer-3 contraction (copies split DVE/ACT to
        # shorten the end-of-kernel critical path)
        h2Tb = act.tile([P, len(MT2), P], F16, tag="h2Tb")
        h2T = []
        for j, (off, sz) in enumerate(MT2):
            pt = pts[j % 3]
            nc.tensor.transpose(pt[:sz, :], h2[:, off:off + sz], ident[:])
            if j % 4 == 3:
                nc.scalar.copy(h2Tb[:sz, j, :], pt[:sz, :])
            else:
                nc.vector.tensor_copy(h2Tb[:sz, j, :], pt[:sz, :])
            h2T.append(h2Tb[:sz, j, :])

        # ---- Layer 3: out = H2 @ W3 + b3
        ps3 = pacc.tile([P, NOUT], F32, tag="ps3")
        for t, (off, sz) in enumerate(MT2):
            nc.tensor.matmul(
                ps3[:],
                lhsT=h2T[t],
                rhs=w3t[:sz, t, :],
                start=(t == 0),
                stop=(t == len(MT2) - 1),
            )
        o = act.tile([P, NOUT], F32, tag="o")
        nc.vector.tensor_add(o[:], ps3[:], b3t[:])
        nc.sync.dma_start(out=out, in_=o[:])


def _get_program():
    if "nc" in _CACHE:
        return _CACHE["nc"]
    nc = bacc.Bacc("TRN2", target_bir_lowering=False, debug=False,
                   enable_asserts=False, num_devices=NCORES)
    xT = nc.dram_tensor("xT", [len(MT1) * P, BS], F16,
                        kind="ExternalInput").ap()
    w1t = nc.dram_tensor("w1t", [NIN, NH], F16, kind="ExternalInput").ap()
    w2t = nc.dram_tensor("w2t", [NH, NH], F16, kind="ExternalInput").ap()
    w3p = nc.dram_tensor("w3p", [P, len(MT2), NOUT], F16,
                         kind="ExternalInput").ap()
    b3r = nc.dram_tensor("b3r", [P, NOUT], F32, kind="ExternalInput").ap()
    out = nc.dram_tensor("out", [BS, NOUT], F32, kind="ExternalOutput").ap()
    with tile.TileContext(nc) as tc:
        _mlp_body(tc, xT, w1t, w2t, w3p, b3r, out)
    nc.compile()
    _CACHE["nc"] = nc
    return nc


def kernel(x, w1, idx1, w2, idx2, masks1, masks2, W3, b3, context):
    global LAST_RESULT
    x = np.ascontiguousarray(np.asarray(x, dtype=np.float32))
    ctxi = int(np.asarray(context))

    weff1 = _build_weff(np.asarray(w1), np.asarray(idx1),
                        np.asarray(masks1)[ctxi], NIN)
    weff2 = _build_weff(np.asarray(w2), np.asarray(idx2),
                        np.asarray(masks2)[ctxi], NH)
    w1t = np.ascontiguousarray(weff1.T.astype(np.float16))    # (784, 2000)
    w2t = np.ascontiguousarray(weff2.T.astype(np.float16))    # (2000, 2000)

    # W3 packed to [128, n_tiles, 10]: w3p[m, t, :] = W3[t*128 + m, :]
    w3f = np.asarray(W3).astype(np.float16)
    w3p = np.zeros((P, len(MT2), NOUT), np.float16)
    for t, (off, sz) in enumerate(MT2):
        w3p[:sz, t, :] = w3f[off:off + sz, :]
    b3r = np.ascontiguousarray(
        np.broadcast_to(np.asarray(b3, dtype=np.float32), (P, NOUT)).copy())

    try:
        import antenv.axon_hooks  # noqa: F401
    except Exception:
        os.environ.setdefault("BASS_NEVER_TRACE", "1")

    nc = _get_program()
    in_maps = []
    for c in range(NCORES):
        xTc = np.zeros((len(MT1) * P, BS), np.float16)
        xTc[:NIN] = x[c * BS:(c + 1) * BS].T.astype(np.float16)
        xT = xTc
        in_maps.append({"xT": xT, "w1t": w1t, "w2t": w2t, "w3p": w3p,
                        "b3r": b3r})

    LAST_RESULT = run_bass_kernel_spmd(nc, in_maps, list(range(NCORES)))
    return np.concatenate(
        [LAST_RESULT.results[c]["out"] for c in range(NCORES)], axis=0)
